# revision 1
# baseline (speedup 1.0000x reference)
"""Multi-head causal attention kernel for Trainium2 (8 NeuronCores).

Problem: B=4, S=2048, HID=1024, H=16 heads (head_dim 64), causal mask,
fp32 I/O.  out = softmax(mask + (XqWq)(XkWk)^T/8) (XvWv) Wo

Sharding: 8 cores = 4 batches x 2 head-groups.  Core c handles batch
c//2 and heads (c%2)*8 .. +8 (dk slice of 512).  Each core computes a
full-shape [S, HID] partial output (its head-group's contribution
through Wo); the host sums the two partials per batch.

Per-core dataflow (all matmuls in float32r = TF32-like, full PE rate):
  - PE-transpose X chunks -> X^T; project to kT (persistent, [e,s]
    layout, 2 heads per 128-partition tile), qT (rotating per-512-q
    window) and v (persistent, natural [s,e] with a ones column per
    head so the PV matmul also emits softmax denominators).
  - Attention in transposed [k,q] orientation per (q-window j, head
    pair): logits^T = kT-chunk (stationary) x qT (moving) with
    causally-restricted columns; additive triangular mask on diagonal
    blocks (DVE, in PSUM); exp on ScalarE PSUM->SBUF; PV accumulates
    ctx^T in PSUM (per-element has_written makes partial-range
    accumulation correct).  Denominator row -> reciprocal (DVE) ->
    partition_broadcast (GpSimd) -> multiply-evacuate ctx^T (DVE).
  - Output projection ctx^T.T @ Wo per q-window, fused into the stream.

The projection work for q-window j+1 is emitted interleaved with the
attention work of window j: the projection matmuls act as PE filler
that keeps the PE HAM activity monitor busy (otherwise the exp-bound
attention inner loop lets the PE clock-gate down to 1.2 GHz).
"""

import numpy as np

B, S, HID = 4, 2048, 1024
H_LOCAL, E_LOCAL = 8, 512  # heads / dk columns handled per core
N_CORES = 8
USE_F32R = True

_cached = {}


def _build():
    from concourse import bacc
    import concourse.bass as bass
    import concourse.mybir as mybir
    import concourse.tile as tile
    from concourse.masks import make_identity

    F32 = mybir.dt.float32
    F32R = mybir.dt.float32r if USE_F32R else mybir.dt.float32
    Exp = mybir.ActivationFunctionType.Exp

    nc = bacc.Bacc()
    xq = nc.dram_tensor("xq", [S, HID], F32R, kind="ExternalInput")
    xk = nc.dram_tensor("xk", [S, HID], F32R, kind="ExternalInput")
    xv = nc.dram_tensor("xv", [S, HID], F32R, kind="ExternalInput")
    wq = nc.dram_tensor("wq", [HID, E_LOCAL], F32R, kind="ExternalInput")
    wk = nc.dram_tensor("wk", [HID, E_LOCAL], F32R, kind="ExternalInput")
    wv = nc.dram_tensor("wv", [HID, E_LOCAL], F32R, kind="ExternalInput")
    wo = nc.dram_tensor("wo", [E_LOCAL, HID], F32R, kind="ExternalInput")
    out = nc.dram_tensor("out", [S, HID], F32, kind="ExternalOutput")

    NST = 8           # projection s-tiles
    STW = S // NST    # 256 rows per s-tile
    NSC = STW // 128  # 2 s-chunks per s-tile
    NDC = HID // 128  # 8 d-chunks
    NEC = E_LOCAL // 128  # 4 e-chunks = head pairs
    NKC = S // 128    # 16 k-chunks
    NQT = 4           # q windows of 512

    with tile.TileContext(nc) as tc:
        with (
            tc.sbuf_pool(name="consts", bufs=1) as consts,
            tc.sbuf_pool(name="persist", bufs=1) as persist,
            tc.sbuf_pool(name="stream", bufs=1) as sm,
            tc.psum_pool(name="ps", bufs=1) as ps,
        ):
            ident_f = consts.tile([128, 128], F32)
            make_identity(nc, ident_f)
            ident = consts.tile([128, 128], F32R)
            nc.vector.tensor_copy(ident, ident_f)
            # additive causal mask for diagonal [k,q] blocks: 0 where
            # k <= q else -1e9
            trimask = consts.tile([128, 128], F32)
            nc.gpsimd.memset(trimask, 0.0)
            nc.gpsimd.affine_select(
                out=trimask, in_=trimask,
                compare_op=mybir.AluOpType.is_ge, fill=-1e9, base=0,
                pattern=[[1, 128]], channel_multiplier=-1,
            )
            ones_col = consts.tile([128, 1], F32)
            nc.vector.memset(ones_col, 1.0)

            kt_sb = [persist.tile([128, S], F32R, name=f"kt{i}", tag=f"kt{i}")
                     for i in range(NEC)]
            v_sb = [persist.tile([128, H_LOCAL, 65], F32R, name=f"v{i}",
                                 tag=f"v{i}") for i in range(NKC)]

            wq_sb = sm.tile([128, NDC, E_LOCAL], F32R, tag="wq", bufs=1)
            wk_sb = sm.tile([128, NDC, E_LOCAL], F32R, tag="wk", bufs=1)
            wv_sb = sm.tile([128, NDC, E_LOCAL], F32R, tag="wv", bufs=1)
            wo_sb = sm.tile([128, NEC, HID], F32R, tag="wo", bufs=1)
            nc.sync.dma_start(
                out=wq_sb, in_=wq.rearrange("(dc p) e -> p dc e", p=128))
            nc.sync.dma_start(
                out=wk_sb, in_=wk.rearrange("(dc p) e -> p dc e", p=128))
            nc.sync.dma_start(
                out=wv_sb, in_=wv.rearrange("(dc p) e -> p dc e", p=128))
            nc.sync.dma_start(
                out=wo_sb, in_=wo.rearrange("(dv p) n -> p dv n", p=128))

            qt_rot = {}   # (window, ec) -> [128, 512] tile
            ctx_rot = {}  # (window, hp) -> [128, 512] tile

            def proj_unit(st, tname):
                """Load + transpose + project one input tensor s-tile."""
                s0 = st * STW
                w = st // 2
                xdram = {"q": xq, "k": xk, "v": xv}[tname]
                xnat = sm.tile([128, NSC, HID], F32R, tag="xnat", bufs=2,
                               name=f"xnat_{tname}{st}")
                nc.sync.dma_start(
                    out=xnat,
                    in_=xdram[s0:s0 + STW, :].rearrange(
                        "(sc p) d -> p sc d", p=128))
                xt = sm.tile([128, NDC, STW], F32R, tag="xt", bufs=2,
                             name=f"xt_{tname}{st}")
                for dcp in range(NDC // 2):
                    tp = ps.tile([128, 512], F32R, tag="work", bufs=2,
                                 name=f"tp_{tname}{st}_{dcp}")
                    for k2 in range(2):
                        dc = dcp * 2 + k2
                        for sc in range(NSC):
                            nc.tensor.transpose(
                                tp[:, k2 * STW + sc * 128:
                                   k2 * STW + (sc + 1) * 128],
                                xnat[:, sc, dc * 128:(dc + 1) * 128],
                                ident)
                    nc.vector.tensor_copy(xt[:, dcp * 2:dcp * 2 + 2, :], tp)

                if tname == "q":
                    for ec in range(NEC):
                        if st % 2 == 0:
                            qt_rot[(w, ec)] = sm.tile(
                                [128, 512], F32R, tag=f"qtr{ec}", bufs=2,
                                name=f"qtr{ec}_{w}")
                        pj = ps.tile([128, STW], F32, tag="work", bufs=2,
                                     name=f"pjq_{st}_{ec}")
                        for dc in range(NDC):
                            nc.tensor.matmul(
                                pj, wq_sb[:, dc, ec * 128:(ec + 1) * 128],
                                xt[:, dc, :],
                                start=(dc == 0), stop=(dc == NDC - 1))
                        off = (st % 2) * STW
                        nc.vector.tensor_copy(
                            qt_rot[(w, ec)][:, off:off + STW], pj)
                elif tname == "k":
                    for ec in range(NEC):
                        pj = ps.tile([128, STW], F32, tag="work", bufs=2,
                                     name=f"pjk_{st}_{ec}")
                        for dc in range(NDC):
                            nc.tensor.matmul(
                                pj, wk_sb[:, dc, ec * 128:(ec + 1) * 128],
                                xt[:, dc, :],
                                start=(dc == 0), stop=(dc == NDC - 1))
                        nc.vector.tensor_copy(
                            kt_sb[ec][:, s0:s0 + STW], pj)
                else:
                    for sc in range(NSC):
                        pv = ps.tile([128, E_LOCAL], F32, tag="work", bufs=2,
                                     name=f"pv_{st}_{sc}")
                        for dc in range(NDC):
                            nc.tensor.matmul(
                                pv, xt[:, dc, sc * 128:(sc + 1) * 128],
                                wv_sb[:, dc, :],
                                start=(dc == 0), stop=(dc == NDC - 1))
                        ci = st * NSC + sc
                        nc.vector.tensor_copy(
                            v_sb[ci][:, :, 0:64],
                            pv.rearrange("p (h e) -> p h e", h=H_LOCAL))
                        ones_b = bass.AP(
                            tensor=ones_col.tensor, offset=ones_col.offset,
                            ap=[ones_col.ap[0], [0, H_LOCAL],
                                ones_col.ap[1]],
                        )
                        nc.vector.tensor_copy(v_sb[ci][:, :, 64:65], ones_b)

            def attention_unit(j, hp):
                q0 = j * 512
                nlast = 4 * j + 3
                qt = qt_rot[(j, hp)]
                cpx = [ps.tile([65, 512], F32, tag="cpx", bufs=2,
                               name=f"cpx{hp}_{j}_{hi}") for hi in range(2)]
                ctx_rot[(j, hp)] = sm.tile([128, 512], F32R, tag=f"ctxr{hp}",
                                           bufs=2, name=f"ctxr{hp}_{j}")
                for c in range(4 * j + 4):
                    vo = max(0, c * 128 - q0)
                    lg = ps.tile([128, 1024], F32, tag="lg", bufs=2,
                                 name=f"lg{hp}_{j}_{c}")
                    pt = sm.tile([128, 1024], F32R, tag="pt", bufs=2,
                                 name=f"pt{hp}_{j}_{c}")
                    for hi in range(2):
                        nc.tensor.matmul(
                            lg[:, hi * 512 + vo:(hi + 1) * 512],
                            kt_sb[hp][hi * 64:(hi + 1) * 64,
                                      c * 128:(c + 1) * 128],
                            qt[hi * 64:(hi + 1) * 64, vo:512],
                            start=True, stop=True)
                    if c >= 4 * j:
                        m = c - 4 * j
                        blk = lg.rearrange("p (hh q) -> p hh q", hh=2)[
                            :, :, m * 128:(m + 1) * 128]
                        tri_b = bass.AP(
                            tensor=trimask.tensor, offset=trimask.offset,
                            ap=[trimask.ap[0], [0, 2], trimask.ap[1]],
                        )
                        nc.vector.tensor_add(blk, blk, tri_b)
                    nc.scalar.activation(pt[:, vo:1024], lg[:, vo:1024], Exp)
                    for hi in range(2):
                        nc.tensor.matmul(
                            cpx[hi][:, vo:512],
                            v_sb[c][:, hp * 2 + hi, :],
                            pt[:, hi * 512 + vo:(hi + 1) * 512],
                            start=(c == 0), stop=(c == nlast))
                for hi in range(2):
                    bcast = sm.tile([64, 512], F32, tag="bcast", bufs=1,
                                    name=f"bc{hp}_{j}_{hi}")
                    nc.vector.tensor_copy(bcast[0:1, :], cpx[hi][64:65, :])
                    nc.vector.reciprocal_approx_fast(
                        out=bcast[0:1, :], in_=bcast[0:1, :])
                    nc.gpsimd.partition_broadcast(bcast, bcast[0:1, :])
                    nc.vector.tensor_mul(
                        ctx_rot[(j, hp)][hi * 64:(hi + 1) * 64, :],
                        cpx[hi][0:64, :], bcast)

            for st in (0, 1):
                for t in ("q", "k", "v"):
                    proj_unit(st, t)

            # Emission = program order: every producer must be emitted
            # before its consumers.  Projection for window j+1 is emitted
            # interleaved with attention(j) as PE filler; attention(3)
            # (the largest window, no projection left) gets the deferred
            # out-projection of window 2 as filler instead.
            fills = {
                0: [(2, "q"), (2, "k"), (2, "v"), (3, "q"), (3, "k"),
                    (3, "v")],
                1: [(4, "q"), (4, "k"), (4, "v"), (5, "q"), (5, "k"),
                    (5, "v")],
                2: [(6, "q"), (6, "k"), (6, "v"), (7, "q"), (7, "k"),
                    (7, "v")],
                3: [],
            }

            def out_block(qc):
                for nh in range(2):
                    po = ps.tile([128, 512], F32, tag="work", bufs=2,
                                 name=f"po{qc}_{nh}")
                    for dvc in range(NEC):
                        nc.tensor.matmul(
                            po,
                            ctx_rot[(qc // 4, dvc)][:,
                                                    (qc % 4) * 128:
                                                    (qc % 4 + 1) * 128],
                            wo_sb[:, dvc, nh * 512:(nh + 1) * 512],
                            start=(dvc == 0), stop=(dvc == NEC - 1))
                    osb = sm.tile([128, 512], F32, tag="osb", bufs=1,
                                  name=f"osb{qc}_{nh}")
                    nc.scalar.copy(osb, po)
                    nc.sync.dma_start(
                        out=out[qc * 128:(qc + 1) * 128,
                                nh * 512:(nh + 1) * 512],
                        in_=osb)

            for j in range(3):
                fill = list(fills[j])
                for hp in range(NEC):
                    attention_unit(j, hp)
                    for _ in range(2):
                        if fill:
                            proj_unit(*fill.pop(0))
                while fill:
                    proj_unit(*fill.pop(0))
                if j < 2:
                    for qc in range(4 * j, 4 * j + 4):
                        out_block(qc)
            # j = 3: out(2) blocks act as the PE filler
            for hp in range(NEC):
                attention_unit(3, hp)
                out_block(8 + hp)
            for qc in range(12, 16):
                out_block(qc)

    nc.compile()
    return nc


def kernel(queries, keys, values, mask=None, Wq=None, Wk=None, Wv=None,
           Wo=None, **_ignored):
    from concourse.bass_utils import run_bass_kernel_spmd

    if "nc" not in _cached:
        _cached["nc"] = _build()
    nc = _cached["nc"]

    scale = np.float32(0.125)  # (DK//H) ** -0.5, exact power of two
    in_maps = []
    for c in range(N_CORES):
        b, g = divmod(c, 2)
        sl = slice(g * E_LOCAL, (g + 1) * E_LOCAL)
        in_maps.append({
            "xq": np.ascontiguousarray(queries[b], dtype=np.float32),
            "xk": np.ascontiguousarray(keys[b], dtype=np.float32),
            "xv": np.ascontiguousarray(values[b], dtype=np.float32),
            "wq": np.ascontiguousarray(Wq[:, sl] * scale),
            "wk": np.ascontiguousarray(Wk[:, sl]),
            "wv": np.ascontiguousarray(Wv[:, sl]),
            "wo": np.ascontiguousarray(Wo[sl, :]),
        })
    res = run_bass_kernel_spmd(nc, in_maps, core_ids=list(range(N_CORES)))
    outs = res.results
    full = np.empty((B, S, HID), np.float32)
    for b in range(B):
        full[b] = outs[2 * b]["out"] + outs[2 * b + 1]["out"]
    return full



# revision 12
# speedup vs baseline: 1.4801x; 1.4801x over previous
"""Multi-head causal attention kernel for Trainium2 (8 NeuronCores).

Problem: B=4, S=2048, HID=1024, H=16 heads (head_dim 64), causal mask,
fp32 I/O.  out = softmax(mask + (XqWq)(XkWk)^T/8) (XvWv) Wo

Sharding: 8 cores = 4 batches x 2 head-groups.  Core c handles batch
c//2 and heads (c%2)*8 .. +8 (dk slice of 512).  Each core computes a
full-shape [S, HID] partial output (its head-group's contribution
through Wo); the host sums the two partials per batch.

Host-side prep: X tensors are transposed to [HID, S] and cast to bf16
(with Wq pre-scaled by 1/8) so the kernel needs no PE transposes and
half the HBM traffic.  All matmul operands are bf16 (1 cycle/column on
the PE vs ~2 for fp32); accumulation stays fp32 in PSUM, mask add and
softmax normalization stay fp32, output is fp32.

Per-core dataflow:
  - Project in s-tiles of 512: qT (per-window, [e,s]), kT (persistent
    [e,s], 2 heads per 128-partition tile), v (persistent [s,e] with a
    ones column per head so the PV matmul also emits softmax
    denominators).
  - Attention in transposed [k,q] orientation per (q-window j, head
    pair hp): logits^T = kT-chunk (stationary) x qT (moving) with
    causally-restricted columns; additive triangular mask on diagonal
    blocks (DVE, in PSUM); exp on ScalarE PSUM->SBUF (bf16 out); PV
    accumulates ctx^T in PSUM.  Denominator rows -> one batched
    reciprocal (DVE) -> partition_broadcast (GpSimd) ->
    multiply-evacuate ctx^T to bf16 (DVE).
  - Output projection ctx^T.T @ Wo per q-window; PSUM evacuation on
    DVE (ScalarE stays dedicated to exp); DMA out in fp32.

Emission order interleaves projection pieces for s-tile j+1 (and the
out-projection of window j) with the attention units of window j, so
the PE always has dense independent matmul work while the exp-bound
attention chain waits on ScalarE, keeping the PE HAM activity monitor
at the full 2.4 GHz clock.
"""

import numpy as np

B, S, HID = 4, 2048, 1024
H_LOCAL, E_LOCAL = 8, 512  # heads / dk columns handled per core
N_CORES = 8

_cached = {}


def _build():
    from concourse import bacc
    import concourse.bass as bass
    import concourse.mybir as mybir
    import concourse.tile as tile

    F32 = mybir.dt.float32
    BF16 = mybir.dt.bfloat16
    Exp = mybir.ActivationFunctionType.Exp

    nc = bacc.Bacc()
    # pre-transposed [HID, S] bf16 inputs
    xq = nc.dram_tensor("xq", [HID, S], BF16, kind="ExternalInput")
    xk = nc.dram_tensor("xk", [HID, S], BF16, kind="ExternalInput")
    xv = nc.dram_tensor("xv", [HID, S], BF16, kind="ExternalInput")
    wq = nc.dram_tensor("wq", [HID, E_LOCAL], BF16, kind="ExternalInput")
    wk = nc.dram_tensor("wk", [HID, E_LOCAL], BF16, kind="ExternalInput")
    wv = nc.dram_tensor("wv", [HID, E_LOCAL], BF16, kind="ExternalInput")
    wo = nc.dram_tensor("wo", [E_LOCAL, HID], BF16, kind="ExternalInput")
    out = nc.dram_tensor("out", [S, HID], F32, kind="ExternalOutput")

    NDC = HID // 128       # 8 d-chunks (contraction)
    NEC = E_LOCAL // 128   # 4 e-chunks = head pairs
    NKC = S // 128         # 16 k-chunks
    NQT = 4                # q windows of 512 = s-tiles
    STW = S // NQT         # 512

    with tile.TileContext(nc) as tc:
        with (
            tc.sbuf_pool(name="consts", bufs=1) as consts,
            tc.sbuf_pool(name="persist", bufs=1) as persist,
            tc.sbuf_pool(name="stream", bufs=1) as sm,
            tc.psum_pool(name="ps", bufs=1) as ps,
        ):
            # additive causal mask for diagonal [k,q] blocks: 0 where
            # k <= q else -1e9
            trimask = consts.tile([128, 128], F32)
            nc.gpsimd.memset(trimask, 0.0)
            nc.gpsimd.affine_select(
                out=trimask, in_=trimask,
                compare_op=mybir.AluOpType.is_ge, fill=-1e9, base=0,
                pattern=[[1, 128]], channel_multiplier=-1,
            )
            ones_col = consts.tile([128, 1], BF16)
            nc.vector.memset(ones_col, 1.0)
            # warm the ACT exp table during the initial DMA wait
            warmup = consts.tile([1, 16], F32)
            nc.vector.memset(warmup, 0.0)
            nc.scalar.activation(warmup, warmup, Exp)

            kt_sb = [persist.tile([128, S], BF16, name=f"kt{i}",
                                  tag=f"kt{i}") for i in range(NEC)]
            v_sb = [persist.tile([128, H_LOCAL, 65], BF16, name=f"v{i}",
                                 tag=f"v{i}") for i in range(NKC)]

            wq_sb = sm.tile([128, NDC, E_LOCAL], BF16, tag="wq", bufs=1)
            wk_sb = sm.tile([128, NDC, E_LOCAL], BF16, tag="wk", bufs=1)
            wv_sb = sm.tile([128, NDC, E_LOCAL], BF16, tag="wv", bufs=1)
            wo_sb = sm.tile([128, NEC, HID], BF16, tag="wo", bufs=1)

            qt_rot = {}   # (window, ec) -> [128, 512] bf16 tile
            ctx_rot = {}  # (window, hp) -> [128, 512] bf16 tile
            xt_tiles = {}  # (tensor, st) -> [128, NDC, 512] tile

            def load_w(which):
                src = {"q": (wq, wq_sb), "k": (wk, wk_sb),
                       "v": (wv, wv_sb)}.get(which)
                if src is not None:
                    nc.sync.dma_start(
                        out=src[1],
                        in_=src[0].rearrange("(dc p) e -> p dc e", p=128))
                else:
                    nc.sync.dma_start(
                        out=wo_sb,
                        in_=wo.rearrange("(dv p) n -> p dv n", p=128))

            def load_xt(tname, st):
                xdram = {"q": xq, "k": xk, "v": xv}[tname]
                t = sm.tile([128, NDC, STW], BF16, tag=f"xt{tname}",
                            bufs=2, name=f"xt_{tname}{st}")
                nc.sync.dma_start(
                    out=t,
                    in_=xdram[:, st * STW:(st + 1) * STW].rearrange(
                        "(dc p) s -> p dc s", p=128))
                xt_tiles[(tname, st)] = t

            def proj_q(st, ec):
                xt = xt_tiles[("q", st)]
                pj = ps.tile([128, STW], F32, tag="work", bufs=2,
                             name=f"pjq_{st}_{ec}")
                for dc in range(NDC):
                    nc.tensor.matmul(
                        pj, wq_sb[:, dc, ec * 128:(ec + 1) * 128],
                        xt[:, dc, :],
                        start=(dc == 0), stop=(dc == NDC - 1))
                qt_rot[(st, ec)] = sm.tile([128, STW], BF16,
                                           tag=f"qtr{ec}", bufs=2,
                                           name=f"qtr{ec}_{st}")
                nc.vector.tensor_copy(qt_rot[(st, ec)], pj)

            def proj_k(st, ec):
                xt = xt_tiles[("k", st)]
                pj = ps.tile([128, STW], F32, tag="work", bufs=2,
                             name=f"pjk_{st}_{ec}")
                for dc in range(NDC):
                    nc.tensor.matmul(
                        pj, wk_sb[:, dc, ec * 128:(ec + 1) * 128],
                        xt[:, dc, :],
                        start=(dc == 0), stop=(dc == NDC - 1))
                nc.vector.tensor_copy(
                    kt_sb[ec][:, st * STW:(st + 1) * STW], pj)

            def proj_v(st, sc):
                xt = xt_tiles[("v", st)]
                pv = ps.tile([128, E_LOCAL], F32, tag="work", bufs=2,
                             name=f"pv_{st}_{sc}")
                for dc in range(NDC):
                    nc.tensor.matmul(
                        pv, xt[:, dc, sc * 128:(sc + 1) * 128],
                        wv_sb[:, dc, :],
                        start=(dc == 0), stop=(dc == NDC - 1))
                ci = st * 4 + sc
                nc.vector.tensor_copy(
                    v_sb[ci][:, :, 0:64],
                    pv.rearrange("p (h e) -> p h e", h=H_LOCAL))
                ones_b = bass.AP(
                    tensor=ones_col.tensor, offset=ones_col.offset,
                    ap=[ones_col.ap[0], [0, H_LOCAL], ones_col.ap[1]],
                )
                nc.vector.tensor_copy(v_sb[ci][:, :, 64:65], ones_b)

            def attention_unit(j, hp):
                q0 = j * 512
                nlast = 4 * j + 3
                qt = qt_rot[(j, hp)]
                cpx = [ps.tile([65, 512], F32, tag="cpx", bufs=2,
                               name=f"cpx{hp}_{j}_{hi}")
                       for hi in range(2)]
                ctx_rot[(j, hp)] = sm.tile([128, 512], BF16,
                                           tag=f"ctxr{hp}", bufs=4,
                                           name=f"ctxr{hp}_{j}")
                for c in range(4 * j + 4):
                    vo = max(0, c * 128 - q0)
                    lg = ps.tile([128, 1024], F32, tag="lg", bufs=2,
                                 name=f"lg{hp}_{j}_{c}")
                    pt = sm.tile([128, 1024], BF16, tag="pt", bufs=3,
                                 name=f"pt{hp}_{j}_{c}")
                    for hi in range(2):
                        nc.tensor.matmul(
                            lg[:, hi * 512 + vo:(hi + 1) * 512],
                            kt_sb[hp][hi * 64:(hi + 1) * 64,
                                      c * 128:(c + 1) * 128],
                            qt[hi * 64:(hi + 1) * 64, vo:512],
                            start=True, stop=True)
                    if c >= 4 * j:
                        m = c - 4 * j
                        blk = lg.rearrange("p (hh q) -> p hh q", hh=2)[
                            :, :, m * 128:(m + 1) * 128]
                        tri_b = bass.AP(
                            tensor=trimask.tensor, offset=trimask.offset,
                            ap=[trimask.ap[0], [0, 2], trimask.ap[1]],
                        )
                        nc.vector.tensor_add(blk, blk, tri_b)
                    if vo == 0:
                        nc.scalar.activation(pt, lg, Exp)
                    else:
                        for hi in range(2):
                            nc.scalar.activation(
                                pt[:, hi * 512 + vo:(hi + 1) * 512],
                                lg[:, hi * 512 + vo:(hi + 1) * 512], Exp)
                    for hi in range(2):
                        nc.tensor.matmul(
                            cpx[hi][:, vo:512],
                            v_sb[c][:, hp * 2 + hi, :],
                            pt[:, hi * 512 + vo:(hi + 1) * 512],
                            start=(c == 0), stop=(c == nlast))
                den = sm.tile([1, 1024], F32, tag="den", bufs=2,
                              name=f"den{hp}_{j}")
                for hi in range(2):
                    nc.vector.tensor_copy(
                        den[0:1, hi * 512:(hi + 1) * 512],
                        cpx[hi][64:65, :])
                nc.vector.reciprocal_approx_fast(out=den, in_=den)
                for hi in range(2):
                    bcast = sm.tile([64, 512], F32, tag=f"bcast{hi}",
                                    bufs=2, name=f"bc{hp}_{j}_{hi}")
                    nc.gpsimd.partition_broadcast(
                        bcast, den[0:1, hi * 512:(hi + 1) * 512])
                    nc.vector.tensor_mul(
                        ctx_rot[(j, hp)][hi * 64:(hi + 1) * 64, :],
                        cpx[hi][0:64, :], bcast)

            def out_block(qc):
                for nh in range(2):
                    po = ps.tile([128, 512], F32, tag="work", bufs=2,
                                 name=f"po{qc}_{nh}")
                    for dvc in range(NEC):
                        nc.tensor.matmul(
                            po,
                            ctx_rot[(qc // 4, dvc)][:,
                                                    (qc % 4) * 128:
                                                    (qc % 4 + 1) * 128],
                            wo_sb[:, dvc, nh * 512:(nh + 1) * 512],
                            start=(dvc == 0), stop=(dvc == NEC - 1))
                    osb = sm.tile([128, 512], F32, tag="osb", bufs=2,
                                  name=f"osb{qc}_{nh}")
                    nc.vector.tensor_copy(osb, po)
                    nc.sync.dma_start(
                        out=out[qc * 128:(qc + 1) * 128,
                                nh * 512:(nh + 1) * 512],
                        in_=osb)

            # ---- emission (= scheduling priority) order ----
            # Prologue: weights + s-tile 0, with attention(0,0)'s
            # dependencies (q0/k0 head-pair 0, all v) first.  Weight
            # loads interleave with x-tile loads so the first
            # projection can start as early as possible.
            load_w("q")
            load_xt("q", 0)
            load_w("k")
            load_xt("k", 0)
            load_w("v")
            load_xt("v", 0)
            load_w("o")
            proj_q(0, 0)
            proj_k(0, 0)
            for sc in range(4):
                proj_v(0, sc)
            pre = [(proj_q, 0, 1), (proj_k, 0, 1),
                   (proj_q, 0, 2), (proj_k, 0, 2),
                   (proj_q, 0, 3), (proj_k, 0, 3)]

            for j in range(NQT):
                # fill pieces: projections for s-tile j+1, ordered so
                # window j+1's early units unblock first
                if j < NQT - 1:
                    st = j + 1
                    loads = [("q", st), ("k", st), ("v", st)]
                    fill = pre + [
                        (proj_q, st, 0), (proj_k, st, 0),
                        (proj_v, st, 0), (proj_v, st, 1),
                        (proj_v, st, 2), (proj_v, st, 3),
                        (proj_q, st, 1), (proj_k, st, 1),
                        (proj_q, st, 2), (proj_k, st, 2),
                        (proj_q, st, 3), (proj_k, st, 3),
                    ]
                    pre = []
                else:
                    loads = []
                    fill = list(pre)
                    pre = []
                per_unit = (len(fill) + 3) // 4 if fill else 0
                for hp in range(NEC):
                    attention_unit(j, hp)
                    if hp == 0:
                        for ld in loads:
                            load_xt(*ld)
                    for _ in range(per_unit):
                        if fill:
                            f = fill.pop(0)
                            f[0](f[1], f[2])
                    if j == NQT - 1:
                        # windows 0..2's output projections are deferred
                        # to here: they are the only independent PE work
                        # left to fill the exp-bound final window
                        for qc in range(4 * hp, 4 * hp + 4):
                            out_block(qc)
            for qc in range(12, 16):
                out_block(qc)

    nc.compile()
    return nc


def _in_maps(queries, keys, values, Wq, Wk, Wv, Wo):
    import ml_dtypes

    bf = ml_dtypes.bfloat16
    scale = np.float32(0.125)  # (DK//H) ** -0.5, exact power of two
    xqt = [np.ascontiguousarray(queries[b].T).astype(bf) for b in range(B)]
    xkt = [np.ascontiguousarray(keys[b].T).astype(bf) for b in range(B)]
    xvt = [np.ascontiguousarray(values[b].T).astype(bf) for b in range(B)]
    in_maps = []
    for c in range(N_CORES):
        b, g = divmod(c, 2)
        sl = slice(g * E_LOCAL, (g + 1) * E_LOCAL)
        in_maps.append({
            "xq": xqt[b],
            "xk": xkt[b],
            "xv": xvt[b],
            "wq": np.ascontiguousarray(Wq[:, sl] * scale).astype(bf),
            "wk": np.ascontiguousarray(Wk[:, sl]).astype(bf),
            "wv": np.ascontiguousarray(Wv[:, sl]).astype(bf),
            "wo": np.ascontiguousarray(Wo[sl, :]).astype(bf),
        })
    return in_maps


def kernel(queries, keys, values, mask=None, Wq=None, Wk=None, Wv=None,
           Wo=None, **_ignored):
    from concourse.bass_utils import run_bass_kernel_spmd

    if "nc" not in _cached:
        _cached["nc"] = _build()
    nc = _cached["nc"]

    in_maps = _in_maps(queries, keys, values, Wq, Wk, Wv, Wo)
    res = run_bass_kernel_spmd(nc, in_maps, core_ids=list(range(N_CORES)))
    outs = res.results
    full = np.empty((B, S, HID), np.float32)
    for b in range(B):
        full[b] = outs[2 * b]["out"] + outs[2 * b + 1]["out"]
    return full


# revision 17
# speedup vs baseline: 1.5244x; 1.0300x over previous
"""Multi-head causal attention kernel for Trainium2 (8 NeuronCores).

Problem: B=4, S=2048, HID=1024, H=16 heads (head_dim 64), causal mask,
fp32 I/O.  out = softmax(mask + (XqWq)(XkWk)^T/8) (XvWv) Wo

Sharding: 8 cores = 4 batches x 2 head-groups.  Core c handles batch
c//2 and heads (c%2)*8 .. +8 (dk slice of 512).  Each core computes a
full-shape [S, HID] partial output (its head-group's contribution
through Wo); the host sums the two partials per batch.

Host-side prep: X tensors are transposed to [HID, S] and cast to bf16
(with Wq pre-scaled by 1/8) so the kernel needs no PE transposes and
half the HBM traffic.  All matmul operands are bf16 (1 cycle/column on
the PE vs ~2 for fp32); accumulation stays fp32 in PSUM, mask add and
softmax normalization stay fp32, output is fp32.

Per-core dataflow:
  - Project in s-tiles of 512: qT (per-window, [e,s]), kT (persistent
    [e,s], 2 heads per 128-partition tile), v (persistent [s,e] with a
    ones column per head so the PV matmul also emits softmax
    denominators).
  - Attention in transposed [k,q] orientation per (q-window j, head
    pair hp): logits^T = kT-chunk (stationary) x qT (moving) with
    causally-restricted columns; additive triangular mask on diagonal
    blocks (DVE, in PSUM); exp on ScalarE PSUM->SBUF (bf16 out); PV
    accumulates ctx^T in PSUM.  Denominator rows -> one batched
    reciprocal (DVE) -> partition_broadcast (GpSimd) ->
    multiply-evacuate ctx^T to bf16 (DVE).
  - Output projection ctx^T.T @ Wo per q-window; PSUM evacuation on
    DVE (ScalarE stays dedicated to exp); DMA out in fp32.

Emission order interleaves projection pieces for s-tile j+1 (and the
out-projection of window j) with the attention units of window j, so
the PE always has dense independent matmul work while the exp-bound
attention chain waits on ScalarE, keeping the PE HAM activity monitor
at the full 2.4 GHz clock.
"""

import numpy as np

B, S, HID = 4, 2048, 1024
H_LOCAL, E_LOCAL = 8, 512  # heads / dk columns handled per core
N_CORES = 8

_cached = {}


def _build():
    from concourse import bacc
    import concourse.bass as bass
    import concourse.mybir as mybir
    import concourse.tile as tile

    F32 = mybir.dt.float32
    BF16 = mybir.dt.bfloat16
    Exp = mybir.ActivationFunctionType.Exp

    nc = bacc.Bacc()
    # pre-transposed [HID, S] bf16 inputs
    xq = nc.dram_tensor("xq", [HID, S], BF16, kind="ExternalInput")
    xk = nc.dram_tensor("xk", [HID, S], BF16, kind="ExternalInput")
    xv = nc.dram_tensor("xv", [HID, S], BF16, kind="ExternalInput")
    wq = nc.dram_tensor("wq", [HID, E_LOCAL], BF16, kind="ExternalInput")
    wk = nc.dram_tensor("wk", [HID, E_LOCAL], BF16, kind="ExternalInput")
    wv = nc.dram_tensor("wv", [HID, E_LOCAL], BF16, kind="ExternalInput")
    wo = nc.dram_tensor("wo", [E_LOCAL, HID], BF16, kind="ExternalInput")
    # bf16 output: halves the store traffic; the host sums the two
    # per-batch partials in fp32 (quantization ~0.2% of partial
    # magnitude, far under the error budget)
    out = nc.dram_tensor("out", [S, HID], BF16, kind="ExternalOutput")

    NDC = HID // 128       # 8 d-chunks (contraction)
    NEC = E_LOCAL // 128   # 4 e-chunks = head pairs
    NKC = S // 128         # 16 k-chunks
    NQT = 4                # q windows of 512 = s-tiles
    STW = S // NQT         # 512

    with tile.TileContext(nc) as tc:
        with (
            tc.sbuf_pool(name="consts", bufs=1) as consts,
            tc.sbuf_pool(name="persist", bufs=1) as persist,
            tc.sbuf_pool(name="stream", bufs=1) as sm,
            tc.psum_pool(name="ps", bufs=1) as ps,
        ):
            # additive causal mask for diagonal [k,q] blocks: 0 where
            # k <= q else -1e9
            trimask = consts.tile([128, 128], F32)
            nc.gpsimd.memset(trimask, 0.0)
            nc.gpsimd.affine_select(
                out=trimask, in_=trimask,
                compare_op=mybir.AluOpType.is_ge, fill=-1e9, base=0,
                pattern=[[1, 128]], channel_multiplier=-1,
            )
            ones_col = consts.tile([128, 1], BF16)
            nc.vector.memset(ones_col, 1.0)
            # warm the ACT exp table during the initial DMA wait
            warmup = consts.tile([1, 16], F32)
            nc.vector.memset(warmup, 0.0)
            nc.scalar.activation(warmup, warmup, Exp)

            kt_sb = [persist.tile([128, S], BF16, name=f"kt{i}",
                                  tag=f"kt{i}") for i in range(NEC)]
            v_sb = [persist.tile([128, H_LOCAL, 65], BF16, name=f"v{i}",
                                 tag=f"v{i}") for i in range(NKC)]

            wq_sb = sm.tile([128, NDC, E_LOCAL], BF16, tag="wq", bufs=1)
            wk_sb = sm.tile([128, NDC, E_LOCAL], BF16, tag="wk", bufs=1)
            wv_sb = sm.tile([128, NDC, E_LOCAL], BF16, tag="wv", bufs=1)
            wo_sb = sm.tile([128, NEC, HID], BF16, tag="wo", bufs=1)

            qt_rot = {}   # (window, ec) -> [128, 512] bf16 tile
            ctx_rot = {}  # (window, hp) -> [128, 512] bf16 tile
            xt_tiles = {}  # (tensor, st) -> [128, NDC, 512] tile

            def load_w(which):
                src = {"q": (wq, wq_sb), "k": (wk, wk_sb),
                       "v": (wv, wv_sb)}.get(which)
                if src is not None:
                    nc.sync.dma_start(
                        out=src[1],
                        in_=src[0].rearrange("(dc p) e -> p dc e", p=128))
                else:
                    nc.sync.dma_start(
                        out=wo_sb,
                        in_=wo.rearrange("(dv p) n -> p dv n", p=128))

            def load_xt(tname, st):
                xdram = {"q": xq, "k": xk, "v": xv}[tname]
                t = sm.tile([128, NDC, STW], BF16, tag=f"xt{tname}",
                            bufs=2, name=f"xt_{tname}{st}")
                nc.sync.dma_start(
                    out=t,
                    in_=xdram[:, st * STW:(st + 1) * STW].rearrange(
                        "(dc p) s -> p dc s", p=128))
                xt_tiles[(tname, st)] = t

            def proj_q(st, ec):
                xt = xt_tiles[("q", st)]
                pj = ps.tile([128, STW], F32, tag="work", bufs=2,
                             name=f"pjq_{st}_{ec}")
                for dc in range(NDC):
                    nc.tensor.matmul(
                        pj, wq_sb[:, dc, ec * 128:(ec + 1) * 128],
                        xt[:, dc, :],
                        start=(dc == 0), stop=(dc == NDC - 1))
                qt_rot[(st, ec)] = sm.tile([128, STW], BF16,
                                           tag=f"qtr{ec}", bufs=2,
                                           name=f"qtr{ec}_{st}")
                nc.vector.tensor_copy(qt_rot[(st, ec)], pj)

            def proj_k(st, ec):
                xt = xt_tiles[("k", st)]
                pj = ps.tile([128, STW], F32, tag="work", bufs=2,
                             name=f"pjk_{st}_{ec}")
                for dc in range(NDC):
                    nc.tensor.matmul(
                        pj, wk_sb[:, dc, ec * 128:(ec + 1) * 128],
                        xt[:, dc, :],
                        start=(dc == 0), stop=(dc == NDC - 1))
                nc.vector.tensor_copy(
                    kt_sb[ec][:, st * STW:(st + 1) * STW], pj)

            def proj_v(st, sc):
                xt = xt_tiles[("v", st)]
                pv = ps.tile([128, E_LOCAL], F32, tag="work", bufs=2,
                             name=f"pv_{st}_{sc}")
                for dc in range(NDC):
                    nc.tensor.matmul(
                        pv, xt[:, dc, sc * 128:(sc + 1) * 128],
                        wv_sb[:, dc, :],
                        start=(dc == 0), stop=(dc == NDC - 1))
                ci = st * 4 + sc
                nc.vector.tensor_copy(
                    v_sb[ci][:, :, 0:64],
                    pv.rearrange("p (h e) -> p h e", h=H_LOCAL))
                ones_b = bass.AP(
                    tensor=ones_col.tensor, offset=ones_col.offset,
                    ap=[ones_col.ap[0], [0, H_LOCAL], ones_col.ap[1]],
                )
                nc.vector.tensor_copy(v_sb[ci][:, :, 64:65], ones_b)

            def attention_unit(j, hp):
                q0 = j * 512
                nlast = 4 * j + 3
                qt = qt_rot[(j, hp)]
                cpx = [ps.tile([65, 512], F32, tag="cpx", bufs=2,
                               name=f"cpx{hp}_{j}_{hi}")
                       for hi in range(2)]
                ctx_rot[(j, hp)] = sm.tile([128, 512], BF16,
                                           tag=f"ctxr{hp}", bufs=4,
                                           name=f"ctxr{hp}_{j}")
                for c in range(4 * j + 4):
                    vo = max(0, c * 128 - q0)
                    lg = ps.tile([128, 1024], F32, tag="lg", bufs=2,
                                 name=f"lg{hp}_{j}_{c}")
                    pt = sm.tile([128, 1024], BF16, tag="pt", bufs=3,
                                 name=f"pt{hp}_{j}_{c}")
                    for hi in range(2):
                        nc.tensor.matmul(
                            lg[:, hi * 512 + vo:(hi + 1) * 512],
                            kt_sb[hp][hi * 64:(hi + 1) * 64,
                                      c * 128:(c + 1) * 128],
                            qt[hi * 64:(hi + 1) * 64, vo:512],
                            start=True, stop=True)
                    if c >= 4 * j:
                        m = c - 4 * j
                        blk = lg.rearrange("p (hh q) -> p hh q", hh=2)[
                            :, :, m * 128:(m + 1) * 128]
                        tri_b = bass.AP(
                            tensor=trimask.tensor, offset=trimask.offset,
                            ap=[trimask.ap[0], [0, 2], trimask.ap[1]],
                        )
                        nc.vector.tensor_add(blk, blk, tri_b)
                    if vo == 0:
                        nc.scalar.activation(pt, lg, Exp)
                    else:
                        for hi in range(2):
                            nc.scalar.activation(
                                pt[:, hi * 512 + vo:(hi + 1) * 512],
                                lg[:, hi * 512 + vo:(hi + 1) * 512], Exp)
                    for hi in range(2):
                        nc.tensor.matmul(
                            cpx[hi][:, vo:512],
                            v_sb[c][:, hp * 2 + hi, :],
                            pt[:, hi * 512 + vo:(hi + 1) * 512],
                            start=(c == 0), stop=(c == nlast))
                den = sm.tile([1, 1024], F32, tag="den", bufs=2,
                              name=f"den{hp}_{j}")
                for hi in range(2):
                    nc.vector.tensor_copy(
                        den[0:1, hi * 512:(hi + 1) * 512],
                        cpx[hi][64:65, :])
                nc.vector.reciprocal_approx_fast(out=den, in_=den)
                for hi in range(2):
                    bcast = sm.tile([64, 512], F32, tag=f"bcast{hi}",
                                    bufs=2, name=f"bc{hp}_{j}_{hi}")
                    nc.gpsimd.partition_broadcast(
                        bcast, den[0:1, hi * 512:(hi + 1) * 512])
                    nc.vector.tensor_mul(
                        ctx_rot[(j, hp)][hi * 64:(hi + 1) * 64, :],
                        cpx[hi][0:64, :], bcast)

            def out_block(qc, on_scalar=False):
                for nh in range(2):
                    po = ps.tile([128, 512], F32, tag="work", bufs=2,
                                 name=f"po{qc}_{nh}")
                    for dvc in range(NEC):
                        nc.tensor.matmul(
                            po,
                            ctx_rot[(qc // 4, dvc)][:,
                                                    (qc % 4) * 128:
                                                    (qc % 4 + 1) * 128],
                            wo_sb[:, dvc, nh * 512:(nh + 1) * 512],
                            start=(dvc == 0), stop=(dvc == NEC - 1))
                    osb = sm.tile([128, 512], BF16, tag="osb", bufs=4,
                                  name=f"osb{qc}_{nh}")
                    if on_scalar:
                        # tail blocks: exp is done, ScalarE is idle
                        nc.scalar.copy(osb, po)
                    else:
                        nc.vector.tensor_copy(osb, po)
                    nc.sync.dma_start(
                        out=out[qc * 128:(qc + 1) * 128,
                                nh * 512:(nh + 1) * 512],
                        in_=osb)

            # ---- emission (= scheduling priority) order ----
            # Prologue: weights + s-tile 0, with attention(0,0)'s
            # dependencies (q0/k0 head-pair 0, all v) first.  Weight
            # loads interleave with x-tile loads so the first
            # projection can start as early as possible.
            # wq + x_q tile 0 load in halves so the first projection's
            # dc 0..3 matmuls start after ~1MB of DMA instead of ~2MB
            xtq0 = sm.tile([128, NDC, STW], BF16, tag="xtq", bufs=2,
                           name="xt_q0")
            xt_tiles[("q", 0)] = xtq0
            for h in range(2):
                dcs = slice(h * 4, h * 4 + 4)
                rows = slice(h * 512, h * 512 + 512)
                nc.sync.dma_start(
                    out=wq_sb[:, dcs, :],
                    in_=wq[rows, :].rearrange("(dc p) e -> p dc e", p=128))
                nc.sync.dma_start(
                    out=xtq0[:, dcs, :],
                    in_=xq[rows, 0:STW].rearrange(
                        "(dc p) s -> p dc s", p=128))
            load_w("k")
            load_xt("k", 0)
            load_w("v")
            load_xt("v", 0)
            load_w("o")
            proj_q(0, 0)
            proj_k(0, 0)
            for sc in range(4):
                proj_v(0, sc)
            pre = [(proj_q, 0, 1), (proj_k, 0, 1),
                   (proj_q, 0, 2), (proj_k, 0, 2),
                   (proj_q, 0, 3), (proj_k, 0, 3)]

            for j in range(NQT):
                # fill pieces: projections for s-tile j+1, ordered so
                # window j+1's early units unblock first
                if j < NQT - 1:
                    st = j + 1
                    loads = [("q", st), ("k", st), ("v", st)]
                    fill = pre + [
                        (proj_q, st, 0), (proj_k, st, 0),
                        (proj_v, st, 0), (proj_v, st, 1),
                        (proj_v, st, 2), (proj_v, st, 3),
                        (proj_q, st, 1), (proj_k, st, 1),
                        (proj_q, st, 2), (proj_k, st, 2),
                        (proj_q, st, 3), (proj_k, st, 3),
                    ]
                    pre = []
                else:
                    loads = []
                    fill = list(pre)
                    pre = []
                per_unit = (len(fill) + 3) // 4 if fill else 0
                for hp in range(NEC):
                    attention_unit(j, hp)
                    if hp == 0:
                        for ld in loads:
                            load_xt(*ld)
                    for _ in range(per_unit):
                        if fill:
                            f = fill.pop(0)
                            f[0](f[1], f[2])
                    if j == NQT - 1:
                        # windows 0..2's output projections are deferred
                        # to here: they are the only independent PE work
                        # left to fill the exp-bound final window
                        for qc in range(4 * hp, 4 * hp + 4):
                            out_block(qc)
            for qc in range(12, 16):
                out_block(qc, on_scalar=True)

    nc.compile()
    return nc


def _in_maps(queries, keys, values, Wq, Wk, Wv, Wo):
    import ml_dtypes

    bf = ml_dtypes.bfloat16
    scale = np.float32(0.125)  # (DK//H) ** -0.5, exact power of two
    xqt = [np.ascontiguousarray(queries[b].T).astype(bf) for b in range(B)]
    xkt = [np.ascontiguousarray(keys[b].T).astype(bf) for b in range(B)]
    xvt = [np.ascontiguousarray(values[b].T).astype(bf) for b in range(B)]
    in_maps = []
    for c in range(N_CORES):
        b, g = divmod(c, 2)
        sl = slice(g * E_LOCAL, (g + 1) * E_LOCAL)
        in_maps.append({
            "xq": xqt[b],
            "xk": xkt[b],
            "xv": xvt[b],
            "wq": np.ascontiguousarray(Wq[:, sl] * scale).astype(bf),
            "wk": np.ascontiguousarray(Wk[:, sl]).astype(bf),
            "wv": np.ascontiguousarray(Wv[:, sl]).astype(bf),
            "wo": np.ascontiguousarray(Wo[sl, :]).astype(bf),
        })
    return in_maps


def kernel(queries, keys, values, mask=None, Wq=None, Wk=None, Wv=None,
           Wo=None, **_ignored):
    from concourse.bass_utils import run_bass_kernel_spmd

    if "nc" not in _cached:
        _cached["nc"] = _build()
    nc = _cached["nc"]

    in_maps = _in_maps(queries, keys, values, Wq, Wk, Wv, Wo)
    res = run_bass_kernel_spmd(nc, in_maps, core_ids=list(range(N_CORES)))
    outs = res.results
    full = np.empty((B, S, HID), np.float32)
    for b in range(B):
        full[b] = (outs[2 * b]["out"].astype(np.float32)
                   + outs[2 * b + 1]["out"].astype(np.float32))
    return full


# revision 19
# speedup vs baseline: 1.5748x; 1.0331x over previous
"""Multi-head causal attention kernel for Trainium2 (8 NeuronCores).

Problem: B=4, S=2048, HID=1024, H=16 heads (head_dim 64), causal mask,
fp32 I/O.  out = softmax(mask + (XqWq)(XkWk)^T/8) (XvWv) Wo

Sharding: 8 cores = 4 batches x 2 head-groups.  Core c handles batch
c//2 and heads (c%2)*8 .. +8 (dk slice of 512).  Each core computes a
full-shape [S, HID] partial output (its head-group's contribution
through Wo); the host sums the two partials per batch.

Host-side prep: X tensors are transposed to [HID, S] and cast to bf16
(with Wq pre-scaled by 1/8) so the kernel needs no PE transposes and
half the HBM traffic.  All matmul operands are bf16 (1 cycle/column on
the PE vs ~2 for fp32); accumulation stays fp32 in PSUM, mask add and
softmax normalization stay fp32, output is fp32.

Per-core dataflow:
  - Project in s-tiles of 512: qT (per-window, [e,s]), kT (persistent
    [e,s], 2 heads per 128-partition tile), v (persistent [s,e] with a
    ones column per head so the PV matmul also emits softmax
    denominators).
  - Attention in transposed [k,q] orientation per (q-window j, head
    pair hp): logits^T = kT-chunk (stationary) x qT (moving) with
    causally-restricted columns; additive triangular mask on diagonal
    blocks (DVE, in PSUM); exp on ScalarE PSUM->SBUF (bf16 out); PV
    accumulates ctx^T in PSUM.  Denominator rows -> one batched
    reciprocal (DVE) -> partition_broadcast (GpSimd) ->
    multiply-evacuate ctx^T to bf16 (DVE).
  - Output projection ctx^T.T @ Wo per q-window; PSUM evacuation on
    DVE (ScalarE stays dedicated to exp); DMA out in fp32.

Emission order interleaves projection pieces for s-tile j+1 (and the
out-projection of window j) with the attention units of window j, so
the PE always has dense independent matmul work while the exp-bound
attention chain waits on ScalarE, keeping the PE HAM activity monitor
at the full 2.4 GHz clock.
"""

import numpy as np

B, S, HID = 4, 2048, 1024
H_LOCAL, E_LOCAL = 8, 512  # heads / dk columns handled per core
N_CORES = 8

_cached = {}


def _build():
    from concourse import bacc
    import concourse.bass as bass
    import concourse.mybir as mybir
    import concourse.tile as tile

    F32 = mybir.dt.float32
    BF16 = mybir.dt.bfloat16
    Exp = mybir.ActivationFunctionType.Exp

    nc = bacc.Bacc()
    # pre-transposed [HID, S] bf16 inputs
    xq = nc.dram_tensor("xq", [HID, S], BF16, kind="ExternalInput")
    xk = nc.dram_tensor("xk", [HID, S], BF16, kind="ExternalInput")
    xv = nc.dram_tensor("xv", [HID, S], BF16, kind="ExternalInput")
    wq = nc.dram_tensor("wq", [HID, E_LOCAL], BF16, kind="ExternalInput")
    wk = nc.dram_tensor("wk", [HID, E_LOCAL], BF16, kind="ExternalInput")
    wv = nc.dram_tensor("wv", [HID, E_LOCAL], BF16, kind="ExternalInput")
    wo = nc.dram_tensor("wo", [E_LOCAL, HID], BF16, kind="ExternalInput")
    # bf16 output: halves the store traffic; the host sums the two
    # per-batch partials in fp32 (quantization ~0.2% of partial
    # magnitude, far under the error budget)
    out = nc.dram_tensor("out", [S, HID], BF16, kind="ExternalOutput")

    NDC = HID // 128       # 8 d-chunks (contraction)
    NEC = E_LOCAL // 128   # 4 e-chunks = head pairs
    NKC = S // 128         # 16 k-chunks
    NQT = 4                # q windows of 512 = s-tiles
    STW = S // NQT         # 512

    with tile.TileContext(nc) as tc:
        with (
            tc.sbuf_pool(name="consts", bufs=1) as consts,
            tc.sbuf_pool(name="persist", bufs=1) as persist,
            tc.sbuf_pool(name="stream", bufs=1) as sm,
            tc.psum_pool(name="ps", bufs=1) as ps,
        ):
            # additive causal mask for diagonal [k,q] blocks: 0 where
            # k <= q else -1e9
            trimask = consts.tile([128, 128], F32)
            nc.gpsimd.memset(trimask, 0.0)
            nc.gpsimd.affine_select(
                out=trimask, in_=trimask,
                compare_op=mybir.AluOpType.is_ge, fill=-1e9, base=0,
                pattern=[[1, 128]], channel_multiplier=-1,
            )
            ones_col = consts.tile([128, 1], BF16)
            nc.vector.memset(ones_col, 1.0)
            # warm the ACT exp table during the initial DMA wait
            warmup = consts.tile([1, 16], F32)
            nc.vector.memset(warmup, 0.0)
            nc.scalar.activation(warmup, warmup, Exp)

            kt_sb = [persist.tile([128, S], BF16, name=f"kt{i}",
                                  tag=f"kt{i}") for i in range(NEC)]
            v_sb = [persist.tile([128, H_LOCAL, 65], BF16, name=f"v{i}",
                                 tag=f"v{i}") for i in range(NKC)]

            wq_sb = sm.tile([128, NDC, E_LOCAL], BF16, tag="wq", bufs=1)
            wk_sb = sm.tile([128, NDC, E_LOCAL], BF16, tag="wk", bufs=1)
            wv_sb = sm.tile([128, NDC, E_LOCAL], BF16, tag="wv", bufs=1)
            wo_sb = sm.tile([128, NEC, HID], BF16, tag="wo", bufs=1)

            qt_rot = {}   # (window, ec) -> [128, 512] bf16 tile
            ctx_rot = {}  # (window, hp) -> [128, 512] bf16 tile
            xt_tiles = {}  # (tensor, st) -> [128, NDC, 512] tile

            def load_w(which):
                src = {"q": (wq, wq_sb), "k": (wk, wk_sb),
                       "v": (wv, wv_sb)}.get(which)
                if src is not None:
                    nc.sync.dma_start(
                        out=src[1],
                        in_=src[0].rearrange("(dc p) e -> p dc e", p=128))
                else:
                    nc.sync.dma_start(
                        out=wo_sb,
                        in_=wo.rearrange("(dv p) n -> p dv n", p=128))

            def load_xt(tname, st):
                xdram = {"q": xq, "k": xk, "v": xv}[tname]
                t = sm.tile([128, NDC, STW], BF16, tag=f"xt{tname}",
                            bufs=2, name=f"xt_{tname}{st}")
                nc.sync.dma_start(
                    out=t,
                    in_=xdram[:, st * STW:(st + 1) * STW].rearrange(
                        "(dc p) s -> p dc s", p=128))
                xt_tiles[(tname, st)] = t

            def proj_q(st, ec):
                xt = xt_tiles[("q", st)]
                pj = ps.tile([128, STW], F32, tag="work", bufs=2,
                             name=f"pjq_{st}_{ec}")
                for dc in range(NDC):
                    nc.tensor.matmul(
                        pj, wq_sb[:, dc, ec * 128:(ec + 1) * 128],
                        xt[:, dc, :],
                        start=(dc == 0), stop=(dc == NDC - 1))
                qt_rot[(st, ec)] = sm.tile([128, STW], BF16,
                                           tag=f"qtr{ec}", bufs=2,
                                           name=f"qtr{ec}_{st}")
                nc.vector.tensor_copy(qt_rot[(st, ec)], pj)

            def proj_k(st, ec):
                xt = xt_tiles[("k", st)]
                pj = ps.tile([128, STW], F32, tag="work", bufs=2,
                             name=f"pjk_{st}_{ec}")
                for dc in range(NDC):
                    nc.tensor.matmul(
                        pj, wk_sb[:, dc, ec * 128:(ec + 1) * 128],
                        xt[:, dc, :],
                        start=(dc == 0), stop=(dc == NDC - 1))
                nc.vector.tensor_copy(
                    kt_sb[ec][:, st * STW:(st + 1) * STW], pj)

            def proj_v(st, sc):
                xt = xt_tiles[("v", st)]
                pv = ps.tile([128, E_LOCAL], F32, tag="work", bufs=2,
                             name=f"pv_{st}_{sc}")
                for dc in range(NDC):
                    nc.tensor.matmul(
                        pv, xt[:, dc, sc * 128:(sc + 1) * 128],
                        wv_sb[:, dc, :],
                        start=(dc == 0), stop=(dc == NDC - 1))
                ci = st * 4 + sc
                nc.vector.tensor_copy(
                    v_sb[ci][:, :, 0:64],
                    pv.rearrange("p (h e) -> p h e", h=H_LOCAL))
                ones_b = bass.AP(
                    tensor=ones_col.tensor, offset=ones_col.offset,
                    ap=[ones_col.ap[0], [0, H_LOCAL], ones_col.ap[1]],
                )
                nc.vector.tensor_copy(v_sb[ci][:, :, 64:65], ones_b)

            def attention_unit(j, hp):
                q0 = j * 512
                nlast = 4 * j + 3
                qt = qt_rot[(j, hp)]
                cpx = [ps.tile([65, 512], F32, tag="cpx", bufs=2,
                               name=f"cpx{hp}_{j}_{hi}")
                       for hi in range(2)]
                ctx_rot[(j, hp)] = sm.tile([128, 512], BF16,
                                           tag=f"ctxr{hp}", bufs=4,
                                           name=f"ctxr{hp}_{j}")
                for c in range(4 * j + 4):
                    vo = max(0, c * 128 - q0)
                    lg = ps.tile([128, 1024], F32, tag="lg", bufs=2,
                                 name=f"lg{hp}_{j}_{c}")
                    pt = sm.tile([128, 1024], BF16, tag="pt", bufs=3,
                                 name=f"pt{hp}_{j}_{c}")
                    for hi in range(2):
                        nc.tensor.matmul(
                            lg[:, hi * 512 + vo:(hi + 1) * 512],
                            kt_sb[hp][hi * 64:(hi + 1) * 64,
                                      c * 128:(c + 1) * 128],
                            qt[hi * 64:(hi + 1) * 64, vo:512],
                            start=True, stop=True)
                    if c >= 4 * j:
                        m = c - 4 * j
                        blk = lg.rearrange("p (hh q) -> p hh q", hh=2)[
                            :, :, m * 128:(m + 1) * 128]
                        tri_b = bass.AP(
                            tensor=trimask.tensor, offset=trimask.offset,
                            ap=[trimask.ap[0], [0, 2], trimask.ap[1]],
                        )
                        nc.vector.tensor_add(blk, blk, tri_b)
                    if vo == 0:
                        nc.scalar.activation(pt, lg, Exp)
                    else:
                        for hi in range(2):
                            nc.scalar.activation(
                                pt[:, hi * 512 + vo:(hi + 1) * 512],
                                lg[:, hi * 512 + vo:(hi + 1) * 512], Exp)
                    for hi in range(2):
                        nc.tensor.matmul(
                            cpx[hi][:, vo:512],
                            v_sb[c][:, hp * 2 + hi, :],
                            pt[:, hi * 512 + vo:(hi + 1) * 512],
                            start=(c == 0), stop=(c == nlast))
                den = sm.tile([1, 1024], F32, tag="den", bufs=2,
                              name=f"den{hp}_{j}")
                for hi in range(2):
                    nc.vector.tensor_copy(
                        den[0:1, hi * 512:(hi + 1) * 512],
                        cpx[hi][64:65, :])
                nc.vector.reciprocal_approx_fast(out=den, in_=den)
                for hi in range(2):
                    bcast = sm.tile([64, 512], F32, tag=f"bcast{hi}",
                                    bufs=2, name=f"bc{hp}_{j}_{hi}")
                    nc.gpsimd.partition_broadcast(
                        bcast, den[0:1, hi * 512:(hi + 1) * 512])
                    nc.vector.tensor_mul(
                        ctx_rot[(j, hp)][hi * 64:(hi + 1) * 64, :],
                        cpx[hi][0:64, :], bcast)

            def out_block(qc, on_scalar=False, dvc_order=(0, 1, 2, 3)):
                for nh in range(2):
                    po = ps.tile([128, 512], F32, tag="work", bufs=2,
                                 name=f"po{qc}_{nh}")
                    for i, dvc in enumerate(dvc_order):
                        nc.tensor.matmul(
                            po,
                            ctx_rot[(qc // 4, dvc)][:,
                                                    (qc % 4) * 128:
                                                    (qc % 4 + 1) * 128],
                            wo_sb[:, dvc, nh * 512:(nh + 1) * 512],
                            start=(i == 0), stop=(i == NEC - 1))
                    osb = sm.tile([128, 512], BF16, tag="osb", bufs=4,
                                  name=f"osb{qc}_{nh}")
                    if on_scalar:
                        # tail blocks: exp is done, ScalarE is idle
                        nc.scalar.copy(osb, po)
                    else:
                        nc.vector.tensor_copy(osb, po)
                    nc.sync.dma_start(
                        out=out[qc * 128:(qc + 1) * 128,
                                nh * 512:(nh + 1) * 512],
                        in_=osb)

            # ---- emission (= scheduling priority) order ----
            # Prologue: weights + s-tile 0, with attention(0,0)'s
            # dependencies (q0/k0 head-pair 0, all v) first.  Weight
            # loads interleave with x-tile loads so the first
            # projection can start as early as possible.
            # wq + x_q tile 0 load in halves so the first projection's
            # dc 0..3 matmuls start after ~1MB of DMA instead of ~2MB
            xtq0 = sm.tile([128, NDC, STW], BF16, tag="xtq", bufs=2,
                           name="xt_q0")
            xt_tiles[("q", 0)] = xtq0
            for h in range(2):
                dcs = slice(h * 4, h * 4 + 4)
                rows = slice(h * 512, h * 512 + 512)
                nc.sync.dma_start(
                    out=wq_sb[:, dcs, :],
                    in_=wq[rows, :].rearrange("(dc p) e -> p dc e", p=128))
                nc.sync.dma_start(
                    out=xtq0[:, dcs, :],
                    in_=xq[rows, 0:STW].rearrange(
                        "(dc p) s -> p dc s", p=128))
            load_w("k")
            load_xt("k", 0)
            load_w("v")
            load_xt("v", 0)
            load_w("o")
            # all q/k projection pieces for s-tile 0 before the first
            # attention unit: the PE stream is in-order, so anything
            # behind attention(0,0) stalls on the x_v DMA otherwise
            for ec in range(NEC):
                proj_q(0, ec)
                proj_k(0, ec)
            for sc in range(4):
                proj_v(0, sc)

            for j in range(NQT):
                if j < NQT - 1:
                    st = j + 1
                    loads = [("q", st), ("k", st), ("v", st)]
                    fill = [
                        (proj_q, st, 0), (proj_k, st, 0),
                        (proj_v, st, 0), (proj_v, st, 1),
                        (proj_v, st, 2), (proj_v, st, 3),
                        (proj_q, st, 1), (proj_k, st, 1),
                        (proj_q, st, 2), (proj_k, st, 2),
                        (proj_q, st, 3), (proj_k, st, 3),
                    ]
                    hp_order = range(NEC)
                else:
                    loads = []
                    fill = []
                    # rotate so head-pair 0 finishes last; the final
                    # out-blocks then order their contraction to put
                    # ctx(3,0) last, hiding the normalization chain
                    hp_order = (1, 2, 3, 0)
                per_unit = (len(fill) + 3) // 4 if fill else 0
                for ui, hp in enumerate(hp_order):
                    attention_unit(j, hp)
                    if ui == 0:
                        for ld in loads:
                            load_xt(*ld)
                    for _ in range(per_unit):
                        if fill:
                            f = fill.pop(0)
                            f[0](f[1], f[2])
                    if j == NQT - 1 and ui < 3:
                        # windows 0..2's output projections, deferred to
                        # here: the only independent PE work left to
                        # fill the exp-bound final window
                        for qc in range(4 * ui, 4 * ui + 4):
                            out_block(qc)
            for qc in range(12, 16):
                out_block(qc, on_scalar=True, dvc_order=(1, 2, 3, 0))

    nc.compile()
    return nc


def _in_maps(queries, keys, values, Wq, Wk, Wv, Wo):
    import ml_dtypes

    bf = ml_dtypes.bfloat16
    scale = np.float32(0.125)  # (DK//H) ** -0.5, exact power of two
    xqt = [np.ascontiguousarray(queries[b].T).astype(bf) for b in range(B)]
    xkt = [np.ascontiguousarray(keys[b].T).astype(bf) for b in range(B)]
    xvt = [np.ascontiguousarray(values[b].T).astype(bf) for b in range(B)]
    in_maps = []
    for c in range(N_CORES):
        b, g = divmod(c, 2)
        sl = slice(g * E_LOCAL, (g + 1) * E_LOCAL)
        in_maps.append({
            "xq": xqt[b],
            "xk": xkt[b],
            "xv": xvt[b],
            "wq": np.ascontiguousarray(Wq[:, sl] * scale).astype(bf),
            "wk": np.ascontiguousarray(Wk[:, sl]).astype(bf),
            "wv": np.ascontiguousarray(Wv[:, sl]).astype(bf),
            "wo": np.ascontiguousarray(Wo[sl, :]).astype(bf),
        })
    return in_maps


def kernel(queries, keys, values, mask=None, Wq=None, Wk=None, Wv=None,
           Wo=None, **_ignored):
    from concourse.bass_utils import run_bass_kernel_spmd

    if "nc" not in _cached:
        _cached["nc"] = _build()
    nc = _cached["nc"]

    in_maps = _in_maps(queries, keys, values, Wq, Wk, Wv, Wo)
    res = run_bass_kernel_spmd(nc, in_maps, core_ids=list(range(N_CORES)))
    outs = res.results
    full = np.empty((B, S, HID), np.float32)
    for b in range(B):
        full[b] = (outs[2 * b]["out"].astype(np.float32)
                   + outs[2 * b + 1]["out"].astype(np.float32))
    return full
